# revision 7
# baseline (speedup 1.0000x reference)
"""Trainium2 Bass kernel for a LISTA layer (nn_ListaLayer).

Reference computation (jax, fp32):
    th = relu(Theta) + 1e-7
    xW = (y @ W) / th
    repeat 16: z = xW + (unit_threshold(z) * th @ S) / th
    out = (unit_threshold(z) * th) @ Dx
where unit_threshold(v) = sign(v) * relu(|v| - 1).

Algebraic restructure (exact): track v = z * th.  Then
    v0 = y @ W
    repeat 16:  u = soft_threshold(v, th) = sign(v) * relu(|v| - th)
                v = v0 + u @ S
    out = soft_threshold(v, th) @ Dx

Delta-form all-fp8 design (validated offline vs fp64):
  X[i] = 1024*v is PERSISTENT in PSUM across all 16 steps (16 tiles of
  [128,256]f32 = all 8 banks).  Each step accumulates only the delta:
      X += fp8(32*du) @ fp8(32*S)          (DoubleRow, 8 instrs/group)
  where du = u_t - u_{t-1}.  No per-step v0 inject, no fp16 steps.
  Elementwise per (i,step): ACT x=X/32 (fp16), DVE c=clip(x,-32th,+32th)
  (one 2-op tensor_scalar), POOL u=x-c (fp16), DVE d8=u-u_prev (fp8).
  soft_threshold(x,32th) == x - clip(x,-32th,32th).

  fp8 error control: the S-quantization bias accumulates as u_acc@R
  (R = 32S - fp8(32S)) and early big-delta quantization noise persists
  in X.  Both are killed by ONE rebase sweep at step 12 (which replaces
  that step's delta sweep):
      memset X; X += eye32 @ (h,l)            (fp8 pair of 32*v0, DR)
               + (uh + ul) @ S8                (fp8 PAIR full-u resync)
               + fp8(2u) @ fp8(16R)            (S-residual correction;
                                                (2u)@(16R) == 32*u@R)
  then steps 13..16 contract the remaining state error.  Offline fp64
  validation: rel err 5.5e-3 (single-fp8 resync fails at 2.4e-2).

  All matmuls use start=False + skip_group_check (PSUM zero regions are
  2KB banks shared by two X tiles, so start=True would poison the
  neighbour's accumulation); X is zeroed by explicit DVE memsets.

  Step matmul sweeps emit each group's high pairs DEFERRED by 2-4 groups
  (pair5 of group g emitted during group g+2, etc.) so the first-emitted
  matmuls never wait on the last d8 tiles of the elementwise sweep --
  removes the per-step PE stall for the shrink chain's tail.

Distribution: data-parallel over batch rows, 8 NeuronCores, 2048 rows
each; W/Theta/S/Dx replicated; no collectives.
"""

import numpy as np
import ml_dtypes
from contextlib import ExitStack

import concourse.bass as bass
import concourse.bacc as bacc
import concourse.tile as tile
import concourse.mybir as mybir
from concourse.bass import ts, ds

P = 128
NCORES = 8
B_FULL, DIN, DD = 16384, 1024, 2048
BSH = B_FULL // NCORES      # 2048 batch rows per core
CH = 256                    # batch columns per chunk (free dim of step matmuls)
NCH = BSH // CH             # 8 chunks
IT = DD // P                # 16 dict tiles
KW = DIN // P               # 8 d_in tiles
CN = 256                    # free dim of phase-C matmuls
SSC = 32.0                  # S pre-scale (denormal-free e4m3)
SC = 32.0                   # u/v scale (v carried at 32x in fp16 views)
NPAIR = IT // 2             # 8 DoubleRow pairs per group
DEFER = {5: 2, 6: 3, 7: 4}  # pair -> groups of emission deferral

F8 = mybir.dt.float8e4
F16 = mybir.dt.float16
F32 = mybir.dt.float32
ADD = mybir.AluOpType.add
SUB = mybir.AluOpType.subtract
MIN = mybir.AluOpType.min
MAX = mybir.AluOpType.max
IDENT = mybir.ActivationFunctionType.Identity
DR = mybir.MatmulPerfMode.DoubleRow

_built = {}


def _rebase_steps(steps: int):
    return (steps - 4,) if steps >= 8 else ()


def _build(steps: int):
    nc = bacc.Bacc("TRN2", target_bir_lowering=False, debug=False, num_devices=NCORES)

    def inp(name, shape, dt):
        return nc.dram_tensor(name, shape, dt, kind="ExternalInput").ap()

    yT_d = inp("yT", (DIN, BSH), F16)       # fp16(y^T)
    W_d = inp("W1024", (DIN, DD), F16)      # fp16(1024*W)
    S8_d = inp("S8", (DD, DD), F8)          # e4m3(32*S)
    S8lo_d = inp("S8lo", (DD, DD), F8)      # e4m3(16*(32S - S8))
    Dx_d = inp("Dx32", (DD, DIN), F16)      # fp16(Dx/32)
    nth_d = inp("nth32", (DD,), F32)        # -32*(relu(Theta)+eps)
    pth_d = inp("pth32", (DD,), F32)        # +32*(relu(Theta)+eps)
    eye2_d = inp("eye2", (P, 2, P), F8)     # (32*I, 32*I) DR pair
    out_d = nc.dram_tensor("out", (BSH, DIN), F32, kind="ExternalOutput").ap()

    rebase_at = _rebase_steps(steps)

    with tile.TileContext(nc) as tc, ExitStack() as top:
        thp = top.enter_context(tc.tile_pool(name="thp", bufs=1))
        nth_t = thp.tile([P, IT], F32)
        pth_t = thp.tile([P, IT], F32)
        eye2_t = thp.tile([P, 2, P], F8)
        nc.sync.dma_start(nth_t[:], nth_d.rearrange("(io p) -> p io", p=P))
        nc.sync.dma_start(pth_t[:], pth_d.rearrange("(io p) -> p io", p=P))
        nc.sync.dma_start(eye2_t[:], eye2_d)

        wpool = top.enter_context(tc.tile_pool(name="wpool", bufs=1))
        spool = top.enter_context(tc.tile_pool(name="spool", bufs=1))
        dxpool = top.enter_context(tc.tile_pool(name="dxpool", bufs=1))
        ypool = top.enter_context(tc.tile_pool(name="ypool", bufs=2))
        upool = top.enter_context(tc.tile_pool(name="upool", bufs=2))
        dpool = top.enter_context(tc.tile_pool(name="dpool", bufs=2))
        vhlp = top.enter_context(tc.tile_pool(name="vhlp", bufs=1))
        rbu = top.enter_context(tc.tile_pool(name="rbu", bufs=1))
        rbl = top.enter_context(tc.tile_pool(name="rbl", bufs=1))
        rbc = top.enter_context(tc.tile_pool(name="rbc", bufs=1))
        xp = top.enter_context(tc.tile_pool(name="xp", bufs=6))
        cp = top.enter_context(tc.tile_pool(name="cp", bufs=6))
        stC = top.enter_context(tc.tile_pool(name="stC", bufs=4))
        psX = top.enter_context(tc.tile_pool(name="psX", bufs=1, space="PSUM"))

        W_t = wpool.tile([P, KW, DD], F16, name="W_t")
        for ko in range(KW):
            nc.sync.dma_start(W_t[:, ko, :], W_d[ts(ko, P), :])
        S8_t = spool.tile([P, IT, DD], F8, name="S8_t")
        for jo in range(IT):
            nc.sync.dma_start(S8_t[:, jo, :], S8_d[ts(jo, P), :])
        S8lo_t = None
        if rebase_at:
            S8lo_t = spool.tile([P, IT, DD], F8, name="S8lo_t")
            for jo in range(IT):
                nc.sync.dma_start(S8lo_t[:, jo, :], S8lo_d[ts(jo, P), :])
        Dx_t = dxpool.tile([P, IT, DIN], F16, name="Dx_t")
        for io in range(IT):
            nc.sync.dma_start(Dx_t[:, io, :], Dx_d[ts(io, P), :])

        y_tiles = []
        for c in range(NCH):
            y_tiles.append(ypool.tile([P, KW, CH], F16, tag="y", name=f"y_{c}"))
        for ko in range(KW):
            nc.sync.dma_start(y_tiles[0][:, ko, :], yT_d[ts(ko, P), ds(0, CH)])

        def mm(out_ap, lhsT, rhs, stop, perf_mode=None):
            nc.tensor.matmul(out_ap, lhsT, rhs, start=False, stop=stop,
                             perf_mode=perf_mode, skip_group_check=True)

        def emit_sweep(X, S_t, rhs, defer=DEFER):
            """DR accumulation sweep: X[g] += S_t[:,:,g-block].T-contract rhs.
            High pairs deferred so early matmuls never wait on late d8."""
            dmax = max(defer.values()) if defer else 0
            for g in range(IT + dmax):
                if g < IT:
                    for p_ in range(NPAIR):
                        if p_ in defer:
                            continue
                        mm(X[g], S_t[:, 2 * p_:2 * p_ + 2, ts(g, P)],
                           rhs[:, 2 * p_:2 * p_ + 2, :], stop=False, perf_mode=DR)
                for p_, dk in defer.items():
                    gg = g - dk
                    if 0 <= gg < IT:
                        mm(X[gg], S_t[:, 2 * p_:2 * p_ + 2, ts(gg, P)],
                           rhs[:, 2 * p_:2 * p_ + 2, :],
                           stop=(p_ == NPAIR - 1), perf_mode=DR)

        for c in range(NCH):
            if c + 1 < NCH:
                for ko in range(KW):
                    nc.sync.dma_start(y_tiles[c + 1][:, ko, :],
                                      yT_d[ts(ko, P), ds((c + 1) * CH, CH)])
            y_c = y_tiles[c]

            # -------- phase A: X[:,i,:] = 1024*v0 (memset + fp16 accumulation)
            # Single [P, IT, CH] f32 tile = exactly all 8 PSUM banks; each
            # [P, CH] slice is half-bank-aligned so matmuls never cross banks.
            X_t = psX.tile([P, IT, CH], F32, tag="psX", name="X_t")
            X = [X_t[:, i, :] for i in range(IT)]
            for i in range(IT):
                nc.vector.memset(X[i], 0.0)
            for i in range(IT):
                for ko in range(KW):
                    mm(X[i], W_t[:, ko, ts(i, P)], y_c[:, ko, :],
                       stop=(ko == KW - 1))

            # -------- steps: t = 1..steps accumulate deltas; steps+1 = final a
            u_prev = None
            v0hl = None
            for t in range(1, steps + 2):
                last = (t == steps + 1)
                rb = (t in rebase_at)
                u_cur = upool.tile([P, IT, CH], F16, tag="u", name="a" if last else "u")
                d8 = None if (last or rb) else dpool.tile([P, IT, CH], F8, tag="d")
                if t == 1 and rebase_at:
                    v0hl = vhlp.tile([P, IT, 2, CH], F8, tag="vhl")
                for i in range(IT):
                    x_t = xp.tile([P, CH], F16, tag="x")
                    nc.scalar.activation(x_t[:], X[i], IDENT, bias=0.0,
                                         scale=1.0 / SSC)
                    if t == 1 and rebase_at:
                        nc.vector.tensor_scalar_add(v0hl[:, i, 0, :], x_t[:], 0.0)
                        nc.gpsimd.tensor_tensor(v0hl[:, i, 1, :], x_t[:],
                                                v0hl[:, i, 0, :], SUB)
                    c_t = cp.tile([P, CH], F16, tag="c")
                    nc.vector.tensor_scalar(c_t[:], x_t[:], nth_t[:, i:i + 1],
                                            pth_t[:, i:i + 1], MAX, MIN)
                    nc.gpsimd.tensor_tensor(u_cur[:, i, :], x_t[:], c_t[:], SUB)
                    if d8 is not None:
                        if t == 1:
                            nc.vector.tensor_scalar_add(d8[:, i, :],
                                                        u_cur[:, i, :], 0.0)
                        else:
                            nc.vector.tensor_tensor(d8[:, i, :], u_cur[:, i, :],
                                                    u_prev[:, i, :], SUB)
                if d8 is not None:
                    emit_sweep(X, S8_t, d8)
                elif rb:
                    # rebase: fresh X = v0(pair) + u(pair)@S8 + (2u)@(16R)
                    uh = rbu.tile([P, IT, CH], F8, tag="uh")
                    ul = rbl.tile([P, IT, CH], F8, tag="ul")
                    c8 = rbc.tile([P, IT, CH], F8, tag="c8")
                    for i in range(IT):
                        nc.vector.tensor_scalar_add(uh[:, i, :],
                                                    u_cur[:, i, :], 0.0)
                        nc.gpsimd.tensor_tensor(ul[:, i, :], u_cur[:, i, :],
                                                uh[:, i, :], SUB)
                        nc.vector.tensor_scalar_mul(c8[:, i, :],
                                                    u_cur[:, i, :], 1.0 / 16.0)
                    for i in range(IT):
                        nc.vector.memset(X[i], 0.0)
                        mm(X[i], eye2_t[:], v0hl[:, i, :, :],
                           stop=False, perf_mode=DR)
                    emit_sweep(X, S8_t, uh)
                    emit_sweep(X, S8_t, ul)
                    emit_sweep(X, S8lo_t, c8)
                u_prev = u_cur

            # -------- phase C: out_chunk = (32a) @ (Dx/32), Dx resident ------
            for dn in range(DIN // CN):
                for bt in range(CH // P):
                    ps = X[dn * (CH // P) + bt]   # reuse freed X slice as psum
                    nc.vector.memset(ps, 0.0)
                    for io in range(IT):
                        mm(ps, u_prev[:, io, ts(bt, P)],
                           Dx_t[:, io, ds(dn * CN, CN)], stop=(io == IT - 1))
                    st = stC.tile([P, CN], F32, tag="stC")
                    nc.scalar.activation(st[:], ps, IDENT, bias=0.0, scale=1.0)
                    nc.sync.dma_start(out_d[ds(c * CH + bt * P, P),
                                            ds(dn * CN, CN)], st[:])

    nc.compile()
    return nc


def _prep_in_maps(y, W, Theta, S, Dx):
    y = np.ascontiguousarray(np.asarray(y, dtype=np.float32))
    W = np.asarray(W, dtype=np.float32)
    Theta = np.asarray(Theta, dtype=np.float32)
    S = np.asarray(S, dtype=np.float32)
    Dx = np.asarray(Dx, dtype=np.float32)
    assert y.shape == (B_FULL, DIN) and W.shape == (DIN, DD)
    assert S.shape == (DD, DD) and Dx.shape == (DD, DIN)

    W1024 = (W * np.float32(SC * SSC)).astype(np.float16)
    S8 = (S * np.float32(SSC)).astype(ml_dtypes.float8_e4m3)
    R = S * np.float32(SSC) - S8.astype(np.float32)
    S8lo = (16.0 * R).astype(ml_dtypes.float8_e4m3)
    Dx32 = (Dx / np.float32(SC)).astype(np.float16)
    th = np.maximum(Theta, 0.0) + np.float32(1e-7)
    nth32 = (-SC * th).astype(np.float32)
    pth32 = (SC * th).astype(np.float32)
    eye32 = (np.eye(P, dtype=np.float32) * 32.0).astype(ml_dtypes.float8_e4m3)
    eye2 = np.ascontiguousarray(np.stack([eye32, eye32], axis=1))  # (P, 2, P)
    yT = np.ascontiguousarray(y.T).astype(np.float16)   # [DIN, B]

    shared = dict(W1024=W1024, S8=S8, S8lo=S8lo, Dx32=Dx32,
                  nth32=nth32, pth32=pth32, eye2=eye2)
    in_maps = []
    for c in range(NCORES):
        sl = slice(c * BSH, (c + 1) * BSH)
        in_maps.append(dict(shared, yT=np.ascontiguousarray(yT[:, sl])))
    return in_maps


_sharded_cache = {}


def _get_sharded(steps: int):
    """Build (once) the jitted shard_map executable for the compiled NEFF."""
    if steps in _sharded_cache:
        return _sharded_cache[steps]
    import jax
    from jax.experimental.shard_map import shard_map
    from jax.sharding import Mesh, PartitionSpec
    from concourse import bass2jax

    if steps not in _built:
        _built[steps] = _build(steps)
    nc = _built[steps]
    bass2jax.install_neuronx_cc_hook()
    assert nc.dbg_addr is None
    partition_name = nc.partition_id_tensor.name if nc.partition_id_tensor else None

    in_names, out_names, out_avals, zero_shapes = [], [], [], []
    for alloc in nc.m.functions[0].allocations:
        if not isinstance(alloc, mybir.MemoryLocationSet):
            continue
        name = alloc.memorylocations[0].name
        if alloc.kind == "ExternalInput":
            if name != partition_name:
                in_names.append(name)
        elif alloc.kind == "ExternalOutput":
            out_names.append(name)
            shape = tuple(alloc.tensor_shape)
            dtype = mybir.dt.np(alloc.dtype)
            out_avals.append(jax.core.ShapedArray(shape, dtype))
            zero_shapes.append((shape, dtype))
    n_params = len(in_names)
    n_outs = len(out_names)
    all_in_names = in_names + out_names
    if partition_name is not None:
        all_in_names.append(partition_name)

    def _body(*args):
        operands = list(args)
        if partition_name is not None:
            operands.append(bass2jax.partition_id_tensor())
        outs = bass2jax._bass_exec_p.bind(
            *operands,
            out_avals=tuple(out_avals),
            in_names=tuple(all_in_names),
            out_names=tuple(out_names),
            lowering_input_output_aliases=(),
            sim_require_finite=True,
            sim_require_nnan=True,
            nc=nc,
        )
        return tuple(outs)

    devices = jax.devices()[:NCORES]
    mesh = Mesh(np.asarray(devices), ("core",))
    donate = tuple(range(n_params, n_params + n_outs))
    sharded = jax.jit(
        shard_map(_body, mesh=mesh,
                  in_specs=(PartitionSpec("core"),) * (n_params + n_outs),
                  out_specs=(PartitionSpec("core"),) * n_outs,
                  check_rep=False),
        donate_argnums=donate, keep_unused=True)
    entry = dict(sharded=sharded, in_names=in_names, out_names=out_names,
                 zero_shapes=zero_shapes, mesh=mesh, n_params=n_params)
    _sharded_cache[steps] = entry
    return entry


def _concat_inputs(entry, in_maps):
    return [np.concatenate([np.asarray(in_maps[c][n]) for c in range(NCORES)], axis=0)
            for n in entry["in_names"]]


def _run(entry, concat_in):
    zeros = [np.zeros((NCORES * s[0], *s[1:]), d) for s, d in entry["zero_shapes"]]
    out_arrs = entry["sharded"](*concat_in, *zeros)
    return out_arrs


def kernel(y, W, Theta, S, Dx, unroll_steps):
    steps = int(unroll_steps)
    entry = _get_sharded(steps)
    in_maps = _prep_in_maps(y, W, Theta, S, Dx)
    out_arrs = _run(entry, _concat_inputs(entry, in_maps))
    idx = entry["out_names"].index("out")
    return np.ascontiguousarray(np.asarray(out_arrs[idx]))  # [NCORES*BSH, DIN]


def time_kernel(np_inputs, iters=6):
    """Steady-state wall time per NEFF execution (ns), device-resident inputs."""
    import jax
    from jax.sharding import NamedSharding, PartitionSpec
    steps = int(np_inputs["unroll_steps"])
    entry = _get_sharded(steps)
    in_maps = _prep_in_maps(np_inputs["y"], np_inputs["W"], np_inputs["Theta"],
                            np_inputs["S"], np_inputs["Dx"])
    concat_in = _concat_inputs(entry, in_maps)
    sh = NamedSharding(entry["mesh"], PartitionSpec("core"))
    dev_in = [jax.device_put(a, sh) for a in concat_in]
    import time as _time
    times = []
    for it in range(iters):
        zeros = [jax.device_put(np.zeros((NCORES * s[0], *s[1:]), d), sh)
                 for s, d in entry["zero_shapes"]]
        for z in zeros:
            z.block_until_ready()
        t0 = _time.perf_counter()
        outs = entry["sharded"](*dev_in, *zeros)
        for o in outs:
            o.block_until_ready()
        times.append(_time.perf_counter() - t0)
    best = min(times[1:]) if len(times) > 1 else times[0]
    print("  per-iter times (ms):", [f"{t*1e3:.1f}" for t in times])
    return best * 1e9


if __name__ == "__main__":
    rng = np.random.default_rng(0)
    inputs = dict(
        y=rng.standard_normal((B_FULL, DIN), dtype=np.float32),
        W=(rng.standard_normal((DIN, DD)) * 0.02).astype(np.float32),
        Theta=rng.random(DD, dtype=np.float32),
        S=(rng.standard_normal((DD, DD)) * 0.02).astype(np.float32),
        Dx=(rng.standard_normal((DD, DIN)) * 0.02).astype(np.float32),
        unroll_steps=16,
    )
    out = kernel(**inputs)
    print("out", out.shape, out.dtype, np.abs(out).max())


# revision 8
# speedup vs baseline: 1.2210x; 1.2210x over previous
"""Trainium2 Bass kernel for a LISTA layer (nn_ListaLayer).

Reference computation (jax, fp32):
    th = relu(Theta) + 1e-7
    xW = (y @ W) / th
    repeat 16: z = xW + (unit_threshold(z) * th @ S) / th
    out = (unit_threshold(z) * th) @ Dx
where unit_threshold(v) = sign(v) * relu(|v| - 1).

Algebraic restructure (exact): track v = z * th.  Then
    v0 = y @ W
    repeat 16:  u = soft_threshold(v, th) = sign(v) * relu(|v| - th)
                v = v0 + u @ S
    out = soft_threshold(v, th) @ Dx

Delta-form all-fp8 design (validated offline vs fp64):
  X[i] = 1024*v is PERSISTENT in PSUM across all 16 steps (16 tiles of
  [128,256]f32 = all 8 banks).  Each step accumulates only the delta:
      X += fp8(32*du) @ fp8(32*S)          (DoubleRow, 8 instrs/group)
  where du = u_t - u_{t-1}.  No per-step v0 inject, no fp16 steps.
  Elementwise per (i,step): ACT x=X/32 (fp16), DVE c=clip(x,-32th,+32th)
  (one 2-op tensor_scalar), POOL u=x-c (fp16), DVE d8=u-u_prev (fp8).
  soft_threshold(x,32th) == x - clip(x,-32th,32th).

  fp8 error control: the S-quantization bias accumulates as u_acc@R
  (R = 32S - fp8(32S)) and early big-delta quantization noise persists
  in X.  Both are killed by ONE rebase sweep at step 12 (which replaces
  that step's delta sweep):
      memset X; X += eye32 @ (h,l)            (fp8 pair of 32*v0, DR)
               + (uh + ul) @ S8                (fp8 PAIR full-u resync)
               + fp8(2u) @ fp8(16R)            (S-residual correction;
                                                (2u)@(16R) == 32*u@R)
  then steps 13..16 contract the remaining state error.  Offline fp64
  validation: rel err 5.5e-3 (single-fp8 resync fails at 2.4e-2).

  All matmuls use start=False + skip_group_check (PSUM zero regions are
  2KB banks shared by two X tiles, so start=True would poison the
  neighbour's accumulation); X is zeroed by explicit DVE memsets.

  Step matmul sweeps emit each group's high pairs DEFERRED by 2-4 groups
  (pair5 of group g emitted during group g+2, etc.) so the first-emitted
  matmuls never wait on the last d8 tiles of the elementwise sweep --
  removes the per-step PE stall for the shrink chain's tail.

Distribution: data-parallel over batch rows, 8 NeuronCores, 2048 rows
each; W/Theta/S/Dx replicated; no collectives.
"""

import numpy as np
import ml_dtypes
from contextlib import ExitStack

import concourse.bass as bass
import concourse.bacc as bacc
import concourse.tile as tile
import concourse.mybir as mybir
from concourse.bass import ts, ds

P = 128
NCORES = 8
B_FULL, DIN, DD = 16384, 1024, 2048
BSH = B_FULL // NCORES      # 2048 batch rows per core
CH = 256                    # batch columns per chunk (free dim of step matmuls)
NCH = BSH // CH             # 8 chunks
IT = DD // P                # 16 dict tiles
KW = DIN // P               # 8 d_in tiles
CN = 256                    # free dim of phase-C matmuls
SSC = 32.0                  # S pre-scale (denormal-free e4m3)
SC = 32.0                   # u/v scale (v carried at 32x in fp16 views)
NPAIR = IT // 2             # 8 DoubleRow pairs per group
DEFER = {5: 2, 6: 3, 7: 4}  # pair -> groups of emission deferral

F8 = mybir.dt.float8e4
F16 = mybir.dt.float16
F32 = mybir.dt.float32
ADD = mybir.AluOpType.add
SUB = mybir.AluOpType.subtract
MIN = mybir.AluOpType.min
MAX = mybir.AluOpType.max
IDENT = mybir.ActivationFunctionType.Identity
DR = mybir.MatmulPerfMode.DoubleRow

_built = {}


def _rebase_steps(steps: int):
    return (steps - 4,) if steps >= 8 else ()


def _build(steps: int):
    nc = bacc.Bacc("TRN2", target_bir_lowering=False, debug=False, num_devices=NCORES)

    def inp(name, shape, dt):
        return nc.dram_tensor(name, shape, dt, kind="ExternalInput").ap()

    yT_d = inp("yT", (DIN, BSH), F16)       # fp16(y^T)
    W_d = inp("W1024", (DIN, DD), F16)      # fp16(1024*W)
    S8_d = inp("S8", (DD, DD), F8)          # e4m3(32*S)
    S8lo_d = inp("S8lo", (DD, DD), F8)      # e4m3(16*(32S - S8))
    Dx_d = inp("Dx32", (DD, DIN), F16)      # fp16(Dx/32)
    nth_d = inp("nth32", (DD,), F32)        # -32*(relu(Theta)+eps)
    pth_d = inp("pth32", (DD,), F32)        # +32*(relu(Theta)+eps)
    eye2_d = inp("eye2", (P, 2, P), F8)     # (32*I, 32*I) DR pair
    out_d = nc.dram_tensor("out", (BSH, DIN), F32, kind="ExternalOutput").ap()

    rebase_at = _rebase_steps(steps)

    with tile.TileContext(nc) as tc, ExitStack() as top:
        thp = top.enter_context(tc.tile_pool(name="thp", bufs=1))
        nth_t = thp.tile([P, IT], F32)
        pth_t = thp.tile([P, IT], F32)
        eye2_t = thp.tile([P, 2, P], F8)
        nc.sync.dma_start(nth_t[:], nth_d.rearrange("(io p) -> p io", p=P))
        nc.sync.dma_start(pth_t[:], pth_d.rearrange("(io p) -> p io", p=P))
        nc.sync.dma_start(eye2_t[:], eye2_d)

        wpool = top.enter_context(tc.tile_pool(name="wpool", bufs=1))
        spool = top.enter_context(tc.tile_pool(name="spool", bufs=1))
        dxpool = top.enter_context(tc.tile_pool(name="dxpool", bufs=1))
        ypool = top.enter_context(tc.tile_pool(name="ypool", bufs=2))
        upool = top.enter_context(tc.tile_pool(name="upool", bufs=2))
        dpool = top.enter_context(tc.tile_pool(name="dpool", bufs=2))
        vhlp = top.enter_context(tc.tile_pool(name="vhlp", bufs=1))
        rbu = top.enter_context(tc.tile_pool(name="rbu", bufs=1))
        rbl = top.enter_context(tc.tile_pool(name="rbl", bufs=1))
        rbc = top.enter_context(tc.tile_pool(name="rbc", bufs=1))
        xp = top.enter_context(tc.tile_pool(name="xp", bufs=6))
        cp = top.enter_context(tc.tile_pool(name="cp", bufs=6))
        stC = top.enter_context(tc.tile_pool(name="stC", bufs=4))
        psX = top.enter_context(tc.tile_pool(name="psX", bufs=1, space="PSUM"))

        W_t = wpool.tile([P, KW, DD], F16, name="W_t")
        for ko in range(KW):
            nc.sync.dma_start(W_t[:, ko, :], W_d[ts(ko, P), :])
        S8_t = spool.tile([P, IT, DD], F8, name="S8_t")
        for jo in range(IT):
            nc.sync.dma_start(S8_t[:, jo, :], S8_d[ts(jo, P), :])
        S8lo_t = None
        if rebase_at:
            S8lo_t = spool.tile([P, IT, DD], F8, name="S8lo_t")
            for jo in range(IT):
                nc.sync.dma_start(S8lo_t[:, jo, :], S8lo_d[ts(jo, P), :])
        Dx_t = dxpool.tile([P, IT, DIN], F16, name="Dx_t")
        for io in range(IT):
            nc.sync.dma_start(Dx_t[:, io, :], Dx_d[ts(io, P), :])

        y_tiles = []
        for c in range(NCH):
            y_tiles.append(ypool.tile([P, KW, CH], F16, tag="y", name=f"y_{c}"))
        for ko in range(KW):
            nc.sync.dma_start(y_tiles[0][:, ko, :], yT_d[ts(ko, P), ds(0, CH)])

        def mm(out_ap, lhsT, rhs, stop, perf_mode=None):
            nc.tensor.matmul(out_ap, lhsT, rhs, start=False, stop=stop,
                             perf_mode=perf_mode, skip_group_check=True)

        def emit_sweep(X, S_t, rhs, defer=DEFER):
            """DR accumulation sweep: X[g] += S_t[:,:,g-block].T-contract rhs.
            High pairs deferred so early matmuls never wait on late d8."""
            dmax = max(defer.values()) if defer else 0
            for g in range(IT + dmax):
                if g < IT:
                    for p_ in range(NPAIR):
                        if p_ in defer:
                            continue
                        mm(X[g], S_t[:, 2 * p_:2 * p_ + 2, ts(g, P)],
                           rhs[:, 2 * p_:2 * p_ + 2, :], stop=False, perf_mode=DR)
                for p_, dk in defer.items():
                    gg = g - dk
                    if 0 <= gg < IT:
                        mm(X[gg], S_t[:, 2 * p_:2 * p_ + 2, ts(gg, P)],
                           rhs[:, 2 * p_:2 * p_ + 2, :],
                           stop=(p_ == NPAIR - 1), perf_mode=DR)

        for c in range(NCH):
            if c + 1 < NCH:
                for ko in range(KW):
                    nc.sync.dma_start(y_tiles[c + 1][:, ko, :],
                                      yT_d[ts(ko, P), ds((c + 1) * CH, CH)])
            y_c = y_tiles[c]

            # -------- phase A: X[:,i,:] = 1024*v0 (memset + fp16 accumulation)
            # Single [P, IT, CH] f32 tile = exactly all 8 PSUM banks; each
            # [P, CH] slice is half-bank-aligned so matmuls never cross banks.
            X_t = psX.tile([P, IT, CH], F32, tag="psX", name="X_t")
            X = [X_t[:, i, :] for i in range(IT)]
            for i in range(IT):
                nc.scalar.activation(X[i], X[i], IDENT, bias=0.0, scale=0.0)
            for i in range(IT):
                for ko in range(KW):
                    mm(X[i], W_t[:, ko, ts(i, P)], y_c[:, ko, :],
                       stop=(ko == KW - 1))

            # -------- steps: t = 1..steps accumulate deltas; steps+1 = final a
            u_prev = None
            v0hl = None
            for t in range(1, steps + 2):
                last = (t == steps + 1)
                rb = (t in rebase_at)
                u_cur = upool.tile([P, IT, CH], F16, tag="u", name="a" if last else "u")
                d8 = None if (last or rb) else dpool.tile([P, IT, CH], F8, tag="d")
                if t == 1 and rebase_at:
                    v0hl = vhlp.tile([P, IT, 2, CH], F8, tag="vhl")
                for i in range(IT):
                    x_t = xp.tile([P, CH], F16, tag="x")
                    nc.scalar.activation(x_t[:], X[i], IDENT, bias=0.0,
                                         scale=1.0 / SSC)
                    if t == 1 and rebase_at:
                        nc.vector.tensor_scalar_add(v0hl[:, i, 0, :], x_t[:], 0.0)
                        nc.gpsimd.tensor_tensor(v0hl[:, i, 1, :], x_t[:],
                                                v0hl[:, i, 0, :], SUB)
                    c_t = cp.tile([P, CH], F16, tag="c")
                    nc.vector.tensor_scalar(c_t[:], x_t[:], nth_t[:, i:i + 1],
                                            pth_t[:, i:i + 1], MAX, MIN)
                    nc.vector.tensor_tensor(u_cur[:, i, :], x_t[:], c_t[:], SUB)
                    if d8 is not None:
                        if t == 1:
                            nc.vector.tensor_scalar_add(d8[:, i, :],
                                                        u_cur[:, i, :], 0.0)
                        else:
                            eng = nc.vector if i % 4 == 0 else nc.gpsimd
                            eng.tensor_tensor(d8[:, i, :], u_cur[:, i, :],
                                              u_prev[:, i, :], SUB)
                if d8 is not None:
                    emit_sweep(X, S8_t, d8)
                elif rb:
                    # rebase: fresh X = v0(pair) + u(pair)@S8 + (2u)@(16R)
                    uh = rbu.tile([P, IT, CH], F8, tag="uh")
                    ul = rbl.tile([P, IT, CH], F8, tag="ul")
                    c8 = rbc.tile([P, IT, CH], F8, tag="c8")
                    for i in range(IT):
                        nc.vector.tensor_scalar_add(uh[:, i, :],
                                                    u_cur[:, i, :], 0.0)
                        nc.gpsimd.tensor_tensor(ul[:, i, :], u_cur[:, i, :],
                                                uh[:, i, :], SUB)
                        nc.vector.tensor_scalar_mul(c8[:, i, :],
                                                    u_cur[:, i, :], 1.0 / 16.0)
                    for i in range(IT):
                        nc.scalar.activation(X[i], X[i], IDENT, bias=0.0,
                                             scale=0.0)
                        mm(X[i], eye2_t[:], v0hl[:, i, :, :],
                           stop=False, perf_mode=DR)
                    emit_sweep(X, S8_t, uh)
                    emit_sweep(X, S8_t, ul)
                    emit_sweep(X, S8lo_t, c8)
                u_prev = u_cur

            # -------- phase C: out_chunk = (32a) @ (Dx/32), Dx resident ------
            for dn in range(DIN // CN):
                for bt in range(CH // P):
                    ps = X[dn * (CH // P) + bt]   # reuse freed X slice as psum
                    nc.scalar.activation(ps, ps, IDENT, bias=0.0, scale=0.0)
                    for io in range(IT):
                        mm(ps, u_prev[:, io, ts(bt, P)],
                           Dx_t[:, io, ds(dn * CN, CN)], stop=(io == IT - 1))
                    st = stC.tile([P, CN], F32, tag="stC")
                    nc.scalar.activation(st[:], ps, IDENT, bias=0.0, scale=1.0)
                    nc.sync.dma_start(out_d[ds(c * CH + bt * P, P),
                                            ds(dn * CN, CN)], st[:])

    nc.compile()
    return nc


def _prep_in_maps(y, W, Theta, S, Dx):
    y = np.ascontiguousarray(np.asarray(y, dtype=np.float32))
    W = np.asarray(W, dtype=np.float32)
    Theta = np.asarray(Theta, dtype=np.float32)
    S = np.asarray(S, dtype=np.float32)
    Dx = np.asarray(Dx, dtype=np.float32)
    assert y.shape == (B_FULL, DIN) and W.shape == (DIN, DD)
    assert S.shape == (DD, DD) and Dx.shape == (DD, DIN)

    W1024 = (W * np.float32(SC * SSC)).astype(np.float16)
    S8 = (S * np.float32(SSC)).astype(ml_dtypes.float8_e4m3)
    R = S * np.float32(SSC) - S8.astype(np.float32)
    S8lo = (16.0 * R).astype(ml_dtypes.float8_e4m3)
    Dx32 = (Dx / np.float32(SC)).astype(np.float16)
    th = np.maximum(Theta, 0.0) + np.float32(1e-7)
    nth32 = (-SC * th).astype(np.float32)
    pth32 = (SC * th).astype(np.float32)
    eye32 = (np.eye(P, dtype=np.float32) * 32.0).astype(ml_dtypes.float8_e4m3)
    eye2 = np.ascontiguousarray(np.stack([eye32, eye32], axis=1))  # (P, 2, P)
    yT = np.ascontiguousarray(y.T).astype(np.float16)   # [DIN, B]

    shared = dict(W1024=W1024, S8=S8, S8lo=S8lo, Dx32=Dx32,
                  nth32=nth32, pth32=pth32, eye2=eye2)
    in_maps = []
    for c in range(NCORES):
        sl = slice(c * BSH, (c + 1) * BSH)
        in_maps.append(dict(shared, yT=np.ascontiguousarray(yT[:, sl])))
    return in_maps


_sharded_cache = {}


def _get_sharded(steps: int):
    """Build (once) the jitted shard_map executable for the compiled NEFF."""
    if steps in _sharded_cache:
        return _sharded_cache[steps]
    import jax
    from jax.experimental.shard_map import shard_map
    from jax.sharding import Mesh, PartitionSpec
    from concourse import bass2jax

    if steps not in _built:
        _built[steps] = _build(steps)
    nc = _built[steps]
    bass2jax.install_neuronx_cc_hook()
    assert nc.dbg_addr is None
    partition_name = nc.partition_id_tensor.name if nc.partition_id_tensor else None

    in_names, out_names, out_avals, zero_shapes = [], [], [], []
    for alloc in nc.m.functions[0].allocations:
        if not isinstance(alloc, mybir.MemoryLocationSet):
            continue
        name = alloc.memorylocations[0].name
        if alloc.kind == "ExternalInput":
            if name != partition_name:
                in_names.append(name)
        elif alloc.kind == "ExternalOutput":
            out_names.append(name)
            shape = tuple(alloc.tensor_shape)
            dtype = mybir.dt.np(alloc.dtype)
            out_avals.append(jax.core.ShapedArray(shape, dtype))
            zero_shapes.append((shape, dtype))
    n_params = len(in_names)
    n_outs = len(out_names)
    all_in_names = in_names + out_names
    if partition_name is not None:
        all_in_names.append(partition_name)

    def _body(*args):
        operands = list(args)
        if partition_name is not None:
            operands.append(bass2jax.partition_id_tensor())
        outs = bass2jax._bass_exec_p.bind(
            *operands,
            out_avals=tuple(out_avals),
            in_names=tuple(all_in_names),
            out_names=tuple(out_names),
            lowering_input_output_aliases=(),
            sim_require_finite=True,
            sim_require_nnan=True,
            nc=nc,
        )
        return tuple(outs)

    devices = jax.devices()[:NCORES]
    mesh = Mesh(np.asarray(devices), ("core",))
    donate = tuple(range(n_params, n_params + n_outs))
    sharded = jax.jit(
        shard_map(_body, mesh=mesh,
                  in_specs=(PartitionSpec("core"),) * (n_params + n_outs),
                  out_specs=(PartitionSpec("core"),) * n_outs,
                  check_rep=False),
        donate_argnums=donate, keep_unused=True)
    entry = dict(sharded=sharded, in_names=in_names, out_names=out_names,
                 zero_shapes=zero_shapes, mesh=mesh, n_params=n_params)
    _sharded_cache[steps] = entry
    return entry


def _concat_inputs(entry, in_maps):
    return [np.concatenate([np.asarray(in_maps[c][n]) for c in range(NCORES)], axis=0)
            for n in entry["in_names"]]


def _run(entry, concat_in):
    zeros = [np.zeros((NCORES * s[0], *s[1:]), d) for s, d in entry["zero_shapes"]]
    out_arrs = entry["sharded"](*concat_in, *zeros)
    return out_arrs


def kernel(y, W, Theta, S, Dx, unroll_steps):
    steps = int(unroll_steps)
    entry = _get_sharded(steps)
    in_maps = _prep_in_maps(y, W, Theta, S, Dx)
    out_arrs = _run(entry, _concat_inputs(entry, in_maps))
    idx = entry["out_names"].index("out")
    return np.ascontiguousarray(np.asarray(out_arrs[idx]))  # [NCORES*BSH, DIN]


def time_kernel(np_inputs, iters=6):
    """Steady-state wall time per NEFF execution (ns), device-resident inputs."""
    import jax
    from jax.sharding import NamedSharding, PartitionSpec
    steps = int(np_inputs["unroll_steps"])
    entry = _get_sharded(steps)
    in_maps = _prep_in_maps(np_inputs["y"], np_inputs["W"], np_inputs["Theta"],
                            np_inputs["S"], np_inputs["Dx"])
    concat_in = _concat_inputs(entry, in_maps)
    sh = NamedSharding(entry["mesh"], PartitionSpec("core"))
    dev_in = [jax.device_put(a, sh) for a in concat_in]
    import time as _time
    times = []
    for it in range(iters):
        zeros = [jax.device_put(np.zeros((NCORES * s[0], *s[1:]), d), sh)
                 for s, d in entry["zero_shapes"]]
        for z in zeros:
            z.block_until_ready()
        t0 = _time.perf_counter()
        outs = entry["sharded"](*dev_in, *zeros)
        for o in outs:
            o.block_until_ready()
        times.append(_time.perf_counter() - t0)
    best = min(times[1:]) if len(times) > 1 else times[0]
    print("  per-iter times (ms):", [f"{t*1e3:.1f}" for t in times])
    return best * 1e9


if __name__ == "__main__":
    rng = np.random.default_rng(0)
    inputs = dict(
        y=rng.standard_normal((B_FULL, DIN), dtype=np.float32),
        W=(rng.standard_normal((DIN, DD)) * 0.02).astype(np.float32),
        Theta=rng.random(DD, dtype=np.float32),
        S=(rng.standard_normal((DD, DD)) * 0.02).astype(np.float32),
        Dx=(rng.standard_normal((DD, DIN)) * 0.02).astype(np.float32),
        unroll_steps=16,
    )
    out = kernel(**inputs)
    print("out", out.shape, out.dtype, np.abs(out).max())


# revision 12
# speedup vs baseline: 1.2455x; 1.0200x over previous
"""Trainium2 Bass kernel for a LISTA layer (nn_ListaLayer).

Reference computation (jax, fp32):
    th = relu(Theta) + 1e-7
    xW = (y @ W) / th
    repeat 16: z = xW + (unit_threshold(z) * th @ S) / th
    out = (unit_threshold(z) * th) @ Dx
where unit_threshold(v) = sign(v) * relu(|v| - 1).

Algebraic restructure (exact): track v = z * th.  Then
    v0 = y @ W
    repeat 16:  u = soft_threshold(v, th) = sign(v) * relu(|v| - th)
                v = v0 + u @ S
    out = soft_threshold(v, th) @ Dx

Delta-form all-fp8 design (validated offline vs fp64):
  X[i] = 1024*v is PERSISTENT in PSUM across all 16 steps (16 tiles of
  [128,256]f32 = all 8 banks).  Each step accumulates only the delta:
      X += fp8(32*du) @ fp8(32*S)          (DoubleRow, 8 instrs/group)
  where du = u_t - u_{t-1}.  No per-step v0 inject, no fp16 steps.
  Elementwise per (i,step): ACT x=X/32 (fp16), DVE c=clip(x,-32th,+32th)
  (one 2-op tensor_scalar), POOL u=x-c (fp16), DVE d8=u-u_prev (fp8).
  soft_threshold(x,32th) == x - clip(x,-32th,32th).

  fp8 error control: the S-quantization bias accumulates as u_acc@R
  (R = 32S - fp8(32S)) and early big-delta quantization noise persists
  in X.  Both are killed by ONE rebase sweep at step 12 (which replaces
  that step's delta sweep):
      memset X; X += eye32 @ (h,l)            (fp8 pair of 32*v0, DR)
               + (uh + ul) @ S8                (fp8 PAIR full-u resync)
               + fp8(2u) @ fp8(16R)            (S-residual correction;
                                                (2u)@(16R) == 32*u@R)
  then steps 13..16 contract the remaining state error.  Offline fp64
  validation: rel err 5.5e-3 (single-fp8 resync fails at 2.4e-2).

  All matmuls use start=False + skip_group_check (PSUM zero regions are
  2KB banks shared by two X tiles, so start=True would poison the
  neighbour's accumulation); X is zeroed by explicit DVE memsets.

  Step matmul sweeps emit each group's high pairs DEFERRED by 2-4 groups
  (pair5 of group g emitted during group g+2, etc.) so the first-emitted
  matmuls never wait on the last d8 tiles of the elementwise sweep --
  removes the per-step PE stall for the shrink chain's tail.

Distribution: data-parallel over batch rows, 8 NeuronCores, 2048 rows
each; W/Theta/S/Dx replicated; no collectives.
"""

import numpy as np
import ml_dtypes
from contextlib import ExitStack

import concourse.bass as bass
import concourse.bacc as bacc
import concourse.tile as tile
import concourse.mybir as mybir
from concourse.bass import ts, ds

P = 128
NCORES = 8
B_FULL, DIN, DD = 16384, 1024, 2048
BSH = B_FULL // NCORES      # 2048 batch rows per core
CH = 256                    # batch columns per chunk (free dim of step matmuls)
NCH = BSH // CH             # 8 chunks
IT = DD // P                # 16 dict tiles
KW = DIN // P               # 8 d_in tiles
CN = 256                    # free dim of phase-C matmuls
SSC = 32.0                  # S pre-scale (denormal-free e4m3)
SC = 32.0                   # u/v scale (v carried at 32x in fp16 views)
NPAIR = IT // 2             # 8 DoubleRow pairs per group
DEFER = {5: 2, 6: 3, 7: 4}  # pair -> groups of emission deferral

F8 = mybir.dt.float8e4
F16 = mybir.dt.float16
F32 = mybir.dt.float32
ADD = mybir.AluOpType.add
SUB = mybir.AluOpType.subtract
MIN = mybir.AluOpType.min
MAX = mybir.AluOpType.max
IDENT = mybir.ActivationFunctionType.Identity
DR = mybir.MatmulPerfMode.DoubleRow

_built = {}


def _rebase_steps(steps: int):
    return (steps - 4,) if steps >= 8 else ()


def _build(steps: int):
    nc = bacc.Bacc("TRN2", target_bir_lowering=False, debug=False, num_devices=NCORES)

    def inp(name, shape, dt):
        return nc.dram_tensor(name, shape, dt, kind="ExternalInput").ap()

    yT_d = inp("yT", (DIN, BSH), F16)       # fp16(y^T)
    W_d = inp("W1024", (DIN, DD), F16)      # fp16(1024*W)
    S8_d = inp("S8", (DD, DD), F8)          # e4m3(32*S)
    S8lo_d = inp("S8lo", (DD, DD), F8)      # e4m3(16*(32S - S8))
    Dx_d = inp("Dx32", (DD, DIN), F16)      # fp16(Dx/32)
    nth_d = inp("nth32", (DD,), F32)        # -32*(relu(Theta)+eps)
    pth_d = inp("pth32", (DD,), F32)        # +32*(relu(Theta)+eps)
    eye2_d = inp("eye2", (P, 2, P), F8)     # (32*I, 32*I) DR pair
    out_d = nc.dram_tensor("out", (BSH, DIN), F32, kind="ExternalOutput").ap()

    rebase_at = _rebase_steps(steps)

    with tile.TileContext(nc) as tc, ExitStack() as top:
        thp = top.enter_context(tc.tile_pool(name="thp", bufs=1))
        nth_t = thp.tile([P, IT], F32)
        pth_t = thp.tile([P, IT], F32)
        eye2_t = thp.tile([P, 2, P], F8)
        nc.sync.dma_start(nth_t[:], nth_d.rearrange("(io p) -> p io", p=P))
        nc.sync.dma_start(pth_t[:], pth_d.rearrange("(io p) -> p io", p=P))
        nc.sync.dma_start(eye2_t[:], eye2_d)

        wpool = top.enter_context(tc.tile_pool(name="wpool", bufs=1))
        spool = top.enter_context(tc.tile_pool(name="spool", bufs=1))
        dxpool = top.enter_context(tc.tile_pool(name="dxpool", bufs=1))
        ypool = top.enter_context(tc.tile_pool(name="ypool", bufs=2))
        upool = top.enter_context(tc.tile_pool(name="upool", bufs=2))
        dpool = top.enter_context(tc.tile_pool(name="dpool", bufs=2))
        vhlp = top.enter_context(tc.tile_pool(name="vhlp", bufs=1))
        rbu = top.enter_context(tc.tile_pool(name="rbu", bufs=1))
        rbl = top.enter_context(tc.tile_pool(name="rbl", bufs=1))
        rbc = top.enter_context(tc.tile_pool(name="rbc", bufs=1))
        xp = top.enter_context(tc.tile_pool(name="xp", bufs=6))
        cp = top.enter_context(tc.tile_pool(name="cp", bufs=6))
        stC = top.enter_context(tc.tile_pool(name="stC", bufs=4))
        psX = top.enter_context(tc.tile_pool(name="psX", bufs=1, space="PSUM"))

        W_t = wpool.tile([P, KW, DD], F16, name="W_t")
        for ko in range(KW):
            nc.sync.dma_start(W_t[:, ko, :], W_d[ts(ko, P), :])
        S8_t = spool.tile([P, IT, DD], F8, name="S8_t")
        for jo in range(IT):
            nc.sync.dma_start(S8_t[:, jo, :], S8_d[ts(jo, P), :])
        S8lo_t = None
        if rebase_at:
            S8lo_t = spool.tile([P, IT, DD], F8, name="S8lo_t")
            for jo in range(IT):
                nc.sync.dma_start(S8lo_t[:, jo, :], S8lo_d[ts(jo, P), :])
        Dx_t = dxpool.tile([P, IT, DIN], F16, name="Dx_t")
        for io in range(IT):
            nc.sync.dma_start(Dx_t[:, io, :], Dx_d[ts(io, P), :])

        y_tiles = []
        for c in range(NCH):
            y_tiles.append(ypool.tile([P, KW, CH], F16, tag="y", name=f"y_{c}"))
        for ko in range(KW):
            nc.sync.dma_start(y_tiles[0][:, ko, :], yT_d[ts(ko, P), ds(0, CH)])

        def mm(out_ap, lhsT, rhs, stop, perf_mode=None, start=False):
            nc.tensor.matmul(out_ap, lhsT, rhs, start=start, stop=stop,
                             perf_mode=perf_mode, skip_group_check=True)

        def emit_sweep(X, S_t, rhs, defer=DEFER, inject=None, fresh=False):
            """DR sweep: X[g] (+)= S-block.T-contract rhs, high pairs deferred
            so early matmuls never wait on the last few rhs tiles.

            fresh=True: rewrite X in place -- even group's first matmul uses
            start=True, which marks the whole 2KB PSUM bank (both slices)
            pending-zero, so each slice's first matmul overwrites (a free
            memset) and the rest accumulate.  inject: v0hl tile whose (h,l)
            fp8 pair opens each group via eye2 (DoubleRow)."""
            dmax = max(defer.values()) if defer else 0
            for g in range(IT + dmax):
                if g < IT:
                    if inject is not None:
                        mm(X[g], eye2_t[:], inject[:, g, :, :], stop=False,
                           perf_mode=DR, start=(fresh and g % 2 == 0))
                    for p_ in range(NPAIR):
                        if p_ in defer:
                            continue
                        mm(X[g], S_t[:, 2 * p_:2 * p_ + 2, ts(g, P)],
                           rhs[:, 2 * p_:2 * p_ + 2, :], stop=False, perf_mode=DR,
                           start=(fresh and inject is None and p_ == 0
                                  and g % 2 == 0))
                for p_, dk in defer.items():
                    gg = g - dk
                    if 0 <= gg < IT:
                        mm(X[gg], S_t[:, 2 * p_:2 * p_ + 2, ts(gg, P)],
                           rhs[:, 2 * p_:2 * p_ + 2, :],
                           stop=(p_ == NPAIR - 1), perf_mode=DR)

        for c in range(NCH):
            if c + 1 < NCH:
                for ko in range(KW):
                    nc.sync.dma_start(y_tiles[c + 1][:, ko, :],
                                      yT_d[ts(ko, P), ds((c + 1) * CH, CH)])
            y_c = y_tiles[c]

            # -------- phase A: X[:,i,:] = 1024*v0 (fp16 accumulation) --------
            # Single [P, IT, CH] f32 tile = exactly all 8 PSUM banks; each
            # [P, CH] slice is half-bank-aligned so matmuls never cross banks.
            # Even group's first matmul start=True = free memset of its bank.
            X_t = psX.tile([P, IT, CH], F32, tag="psX", name="X_t")
            X = [X_t[:, i, :] for i in range(IT)]
            for i in range(IT):
                for ko in range(KW):
                    mm(X[i], W_t[:, ko, ts(i, P)], y_c[:, ko, :],
                       stop=(ko == KW - 1), start=(ko == 0 and i % 2 == 0))

            # -------- steps: fresh (t<rb), rebase (t=rb), delta tail, final a
            rb_t = rebase_at[0] if rebase_at else None
            u_prev = None
            v0hl = None
            for t in range(1, steps + 2):
                last = (t == steps + 1)
                rb = (t == rb_t)
                fresh = (rb_t is not None) and (t < rb_t)
                delta = not (last or rb or fresh)
                u16 = None
                if rb or delta or last:
                    u16 = upool.tile([P, IT, CH], F16, tag="u",
                                     name="a" if last else "u")
                u8 = None if not fresh else dpool.tile([P, IT, CH], F8, tag="d")
                d8 = None if not delta else dpool.tile([P, IT, CH], F8, tag="d")
                if t == 1:
                    v0hl = vhlp.tile([P, IT, 2, CH], F8, tag="vhl")
                for i in range(IT):
                    x_t = xp.tile([P, CH], F16, tag="x")
                    nc.scalar.activation(x_t[:], X[i], IDENT, bias=0.0,
                                         scale=1.0 / SSC)
                    if t == 1:
                        nc.vector.tensor_scalar_add(v0hl[:, i, 0, :], x_t[:], 0.0)
                        eng = nc.gpsimd if i % 2 == 0 else nc.vector
                        eng.tensor_tensor(v0hl[:, i, 1, :], x_t[:],
                                          v0hl[:, i, 0, :], SUB)
                    c_t = cp.tile([P, CH], F16, tag="c")
                    nc.vector.tensor_scalar(c_t[:], x_t[:], nth_t[:, i:i + 1],
                                            pth_t[:, i:i + 1], MAX, MIN)
                    if fresh:
                        # u8 = fp8(x - c) directly, split DVE/POOL
                        eng = nc.vector if i % 2 == 0 else nc.gpsimd
                        eng.tensor_tensor(u8[:, i, :], x_t[:], c_t[:], SUB)
                    else:
                        nc.vector.tensor_tensor(u16[:, i, :], x_t[:], c_t[:], SUB)
                        if delta:
                            if u_prev is None:   # steps<8: first step is a cast
                                nc.vector.tensor_scalar_add(d8[:, i, :],
                                                            u16[:, i, :], 0.0)
                            else:
                                eng = nc.vector if i % 3 == 0 else nc.gpsimd
                                eng.tensor_tensor(d8[:, i, :], u16[:, i, :],
                                                  u_prev[:, i, :], SUB)
                if fresh:
                    emit_sweep(X, S8_t, u8, inject=v0hl, fresh=True)
                elif delta:
                    emit_sweep(X, S8_t, d8)
                elif rb:
                    # rebase: fresh X = v0(pair) + u(pair)@S8 + (2u)@(16R)
                    uh = rbu.tile([P, IT, CH], F8, tag="uh")
                    ul = rbl.tile([P, IT, CH], F8, tag="ul")
                    c8 = rbc.tile([P, IT, CH], F8, tag="c8")
                    for i in range(IT):
                        nc.vector.tensor_scalar_add(uh[:, i, :], u16[:, i, :], 0.0)
                        nc.gpsimd.tensor_tensor(ul[:, i, :], u16[:, i, :],
                                                uh[:, i, :], SUB)
                        nc.vector.tensor_scalar_mul(c8[:, i, :], u16[:, i, :],
                                                    1.0 / 16.0)
                    emit_sweep(X, S8_t, uh, inject=v0hl, fresh=True)
                    emit_sweep(X, S8_t, ul)
                    emit_sweep(X, S8lo_t, c8)
                if u16 is not None:
                    u_prev = u16

            # -------- phase C: out_chunk = (32a) @ (Dx/32), Dx resident ------
            for dn in range(DIN // CN):
                for bt in range(CH // P):
                    q = dn * (CH // P) + bt
                    ps = X[q]                     # reuse freed X slice as psum
                    for io in range(IT):
                        mm(ps, u_prev[:, io, ts(bt, P)],
                           Dx_t[:, io, ds(dn * CN, CN)], stop=(io == IT - 1),
                           start=(io == 0 and q % 2 == 0))
                    st = stC.tile([P, CN], F32, tag="stC")
                    nc.scalar.activation(st[:], ps, IDENT, bias=0.0, scale=1.0)
                    nc.sync.dma_start(out_d[ds(c * CH + bt * P, P),
                                            ds(dn * CN, CN)], st[:])

    nc.compile()
    return nc


def _prep_in_maps(y, W, Theta, S, Dx):
    y = np.ascontiguousarray(np.asarray(y, dtype=np.float32))
    W = np.asarray(W, dtype=np.float32)
    Theta = np.asarray(Theta, dtype=np.float32)
    S = np.asarray(S, dtype=np.float32)
    Dx = np.asarray(Dx, dtype=np.float32)
    assert y.shape == (B_FULL, DIN) and W.shape == (DIN, DD)
    assert S.shape == (DD, DD) and Dx.shape == (DD, DIN)

    W1024 = (W * np.float32(SC * SSC)).astype(np.float16)
    S8 = (S * np.float32(SSC)).astype(ml_dtypes.float8_e4m3)
    R = S * np.float32(SSC) - S8.astype(np.float32)
    S8lo = (16.0 * R).astype(ml_dtypes.float8_e4m3)
    Dx32 = (Dx / np.float32(SC)).astype(np.float16)
    th = np.maximum(Theta, 0.0) + np.float32(1e-7)
    nth32 = (-SC * th).astype(np.float32)
    pth32 = (SC * th).astype(np.float32)
    eye32 = (np.eye(P, dtype=np.float32) * 32.0).astype(ml_dtypes.float8_e4m3)
    eye2 = np.ascontiguousarray(np.stack([eye32, eye32], axis=1))  # (P, 2, P)
    yT = np.ascontiguousarray(y.T).astype(np.float16)   # [DIN, B]

    shared = dict(W1024=W1024, S8=S8, S8lo=S8lo, Dx32=Dx32,
                  nth32=nth32, pth32=pth32, eye2=eye2)
    in_maps = []
    for c in range(NCORES):
        sl = slice(c * BSH, (c + 1) * BSH)
        in_maps.append(dict(shared, yT=np.ascontiguousarray(yT[:, sl])))
    return in_maps


_sharded_cache = {}


def _get_sharded(steps: int):
    """Build (once) the jitted shard_map executable for the compiled NEFF."""
    if steps in _sharded_cache:
        return _sharded_cache[steps]
    import jax
    from jax.experimental.shard_map import shard_map
    from jax.sharding import Mesh, PartitionSpec
    from concourse import bass2jax

    if steps not in _built:
        _built[steps] = _build(steps)
    nc = _built[steps]
    bass2jax.install_neuronx_cc_hook()
    assert nc.dbg_addr is None
    partition_name = nc.partition_id_tensor.name if nc.partition_id_tensor else None

    in_names, out_names, out_avals, zero_shapes = [], [], [], []
    for alloc in nc.m.functions[0].allocations:
        if not isinstance(alloc, mybir.MemoryLocationSet):
            continue
        name = alloc.memorylocations[0].name
        if alloc.kind == "ExternalInput":
            if name != partition_name:
                in_names.append(name)
        elif alloc.kind == "ExternalOutput":
            out_names.append(name)
            shape = tuple(alloc.tensor_shape)
            dtype = mybir.dt.np(alloc.dtype)
            out_avals.append(jax.core.ShapedArray(shape, dtype))
            zero_shapes.append((shape, dtype))
    n_params = len(in_names)
    n_outs = len(out_names)
    all_in_names = in_names + out_names
    if partition_name is not None:
        all_in_names.append(partition_name)

    def _body(*args):
        operands = list(args)
        if partition_name is not None:
            operands.append(bass2jax.partition_id_tensor())
        outs = bass2jax._bass_exec_p.bind(
            *operands,
            out_avals=tuple(out_avals),
            in_names=tuple(all_in_names),
            out_names=tuple(out_names),
            lowering_input_output_aliases=(),
            sim_require_finite=True,
            sim_require_nnan=True,
            nc=nc,
        )
        return tuple(outs)

    devices = jax.devices()[:NCORES]
    mesh = Mesh(np.asarray(devices), ("core",))
    donate = tuple(range(n_params, n_params + n_outs))
    sharded = jax.jit(
        shard_map(_body, mesh=mesh,
                  in_specs=(PartitionSpec("core"),) * (n_params + n_outs),
                  out_specs=(PartitionSpec("core"),) * n_outs,
                  check_rep=False),
        donate_argnums=donate, keep_unused=True)
    entry = dict(sharded=sharded, in_names=in_names, out_names=out_names,
                 zero_shapes=zero_shapes, mesh=mesh, n_params=n_params)
    _sharded_cache[steps] = entry
    return entry


def _concat_inputs(entry, in_maps):
    return [np.concatenate([np.asarray(in_maps[c][n]) for c in range(NCORES)], axis=0)
            for n in entry["in_names"]]


def _run(entry, concat_in):
    zeros = [np.zeros((NCORES * s[0], *s[1:]), d) for s, d in entry["zero_shapes"]]
    out_arrs = entry["sharded"](*concat_in, *zeros)
    return out_arrs


def kernel(y, W, Theta, S, Dx, unroll_steps):
    steps = int(unroll_steps)
    entry = _get_sharded(steps)
    in_maps = _prep_in_maps(y, W, Theta, S, Dx)
    out_arrs = _run(entry, _concat_inputs(entry, in_maps))
    idx = entry["out_names"].index("out")
    return np.ascontiguousarray(np.asarray(out_arrs[idx]))  # [NCORES*BSH, DIN]


def time_kernel(np_inputs, iters=6):
    """Steady-state wall time per NEFF execution (ns), device-resident inputs."""
    import jax
    from jax.sharding import NamedSharding, PartitionSpec
    steps = int(np_inputs["unroll_steps"])
    entry = _get_sharded(steps)
    in_maps = _prep_in_maps(np_inputs["y"], np_inputs["W"], np_inputs["Theta"],
                            np_inputs["S"], np_inputs["Dx"])
    concat_in = _concat_inputs(entry, in_maps)
    sh = NamedSharding(entry["mesh"], PartitionSpec("core"))
    dev_in = [jax.device_put(a, sh) for a in concat_in]
    import time as _time
    times = []
    for it in range(iters):
        zeros = [jax.device_put(np.zeros((NCORES * s[0], *s[1:]), d), sh)
                 for s, d in entry["zero_shapes"]]
        for z in zeros:
            z.block_until_ready()
        t0 = _time.perf_counter()
        outs = entry["sharded"](*dev_in, *zeros)
        for o in outs:
            o.block_until_ready()
        times.append(_time.perf_counter() - t0)
    best = min(times[1:]) if len(times) > 1 else times[0]
    print("  per-iter times (ms):", [f"{t*1e3:.1f}" for t in times])
    return best * 1e9


if __name__ == "__main__":
    rng = np.random.default_rng(0)
    inputs = dict(
        y=rng.standard_normal((B_FULL, DIN), dtype=np.float32),
        W=(rng.standard_normal((DIN, DD)) * 0.02).astype(np.float32),
        Theta=rng.random(DD, dtype=np.float32),
        S=(rng.standard_normal((DD, DD)) * 0.02).astype(np.float32),
        Dx=(rng.standard_normal((DD, DIN)) * 0.02).astype(np.float32),
        unroll_steps=16,
    )
    out = kernel(**inputs)
    print("out", out.shape, out.dtype, np.abs(out).max())


# revision 23
# speedup vs baseline: 1.4356x; 1.1527x over previous
"""Trainium2 Bass kernel for a LISTA layer (nn_ListaLayer).

Reference computation (jax, fp32):
    th = relu(Theta) + 1e-7
    xW = (y @ W) / th
    repeat 16: z = xW + (unit_threshold(z) * th @ S) / th
    out = (unit_threshold(z) * th) @ Dx
where unit_threshold(v) = sign(v) * relu(|v| - 1).

Algebraic restructure (exact): track v = z * th.  Then
    v0 = y @ W
    repeat 16:  u = soft_threshold(v, th) = sign(v) * relu(|v| - th)
                v = v0 + u @ S
    out = soft_threshold(v, th) @ Dx

All-fp8 hybrid design (validated offline vs fp64, rel err 5.9e-3):
  X[i] = 1024*v lives in PSUM, one [128,16,256]f32 tile = all 8 banks.
  All 17 shrinks use the clip identity soft_threshold(x,32th) ==
  x - clip(x,-32th,32th): ACT x=X/32 (fp16, reads psum), DVE
  c=clip(x,...) (one 2-op tensor_scalar), then ONE combine op.

  Steps 1..11 FRESH: u8 = fp8(x-c) directly (DVE/POOL split); X is
  REWRITTEN each step as eye32@(h,l) [fp8 pair of 32*v0, DoubleRow
  inject] + u8 @ fp8(32*S) [8 DR pairs].  The per-bank overwrite is
  free: the even slice's first matmul uses start=True, which marks the
  whole 2KB bank pending-zero, so each slice's first matmul overwrites
  (no memset) and the rest accumulate; skip_group_check silences the
  group checker.

  Step 12 REBASE kills the accumulated S-quantization bias u@R
  (R = 32S - fp8(32S)) and all fresh-step fp8 noise in one sweep:
      X = eye32@(h,l) + (uh+ul)@S8 + fp8(2u)@fp8(16R)
  (uh,ul = fp8 PAIR of 32u -- single-fp8 resync fails at 2.4e-2;
  (2u)@(16R) == 32*u@R cancels the S-residual exactly).

  Steps 13..16 DELTA: X += fp8(32*(u_t - u_{t-1}))@S8 accumulates only
  the shrinking delta, so late-step fp8 noise is O(|du|), and the
  remaining state error contracts through the tail.

  Scheduling: per-step matmul sweeps emit the first DEFER_G groups'
  high pairs DEFER_SHIFT group-slots late, so the sweep's earliest
  demand for the last u8 tiles lands ~1.5us into the sweep while late
  groups still complete early; the elementwise sweep processes tiles
  in the matmul sweep's completion order, with the last-completing
  tiles' combine op routed to DVE (shorter chain than POOL).  Phase C
  reuses freed X slices as its psum (subtile deps order it after the
  final shrink's reads).

Distribution: data-parallel over batch rows, 8 NeuronCores, 2048 rows
each; W/Theta/S/Dx replicated; no collectives.
"""

import numpy as np
import ml_dtypes
from contextlib import ExitStack

import concourse.bass as bass
import concourse.bacc as bacc
import concourse.tile as tile
import concourse.mybir as mybir
from concourse.bass import ts, ds

P = 128
NCORES = 8
B_FULL, DIN, DD = 16384, 1024, 2048
BSH = B_FULL // NCORES      # 2048 batch rows per core
CH = 256                    # batch columns per chunk (free dim of step matmuls)
NCH = BSH // CH             # 8 chunks
IT = DD // P                # 16 dict tiles
KW = DIN // P               # 8 d_in tiles
CN = 256                    # free dim of phase-C matmuls
SSC = 32.0                  # S pre-scale (denormal-free e4m3)
SC = 32.0                   # u/v scale (v carried at 32x in fp16 views)
NPAIR = IT // 2             # 8 DoubleRow pairs per group
import os as _os
DEFER = eval(_os.environ.get("KM_DEFER", "{5: 2, 6: 3, 7: 4}"))  # pair -> emission deferral

F8 = mybir.dt.float8e4
F16 = mybir.dt.float16
F32 = mybir.dt.float32
ADD = mybir.AluOpType.add
SUB = mybir.AluOpType.subtract
MIN = mybir.AluOpType.min
MAX = mybir.AluOpType.max
IDENT = mybir.ActivationFunctionType.Identity
DR = mybir.MatmulPerfMode.DoubleRow

_built = {}


def _rebase_steps(steps: int):
    return (steps - 4,) if steps >= 8 else ()


def _build(steps: int):
    nc = bacc.Bacc("TRN2", target_bir_lowering=False, debug=False, num_devices=NCORES)

    def inp(name, shape, dt):
        return nc.dram_tensor(name, shape, dt, kind="ExternalInput").ap()

    yT_d = inp("yT", (DIN, BSH), F16)       # fp16(y^T)
    W_d = inp("W1024", (DIN, DD), F16)      # fp16(1024*W)
    S8_d = inp("S8", (DD, DD), F8)          # e4m3(32*S)
    S8lo_d = inp("S8lo", (DD, DD), F8)      # e4m3(16*(32S - S8))
    Dx_d = inp("Dx32", (DD, DIN), F16)      # fp16(Dx/32)
    nth_d = inp("nth32", (DD,), F32)        # -32*(relu(Theta)+eps)
    pth_d = inp("pth32", (DD,), F32)        # +32*(relu(Theta)+eps)
    eye2_d = inp("eye2", (P, 2, P), F8)     # (32*I, 32*I) DR pair
    out_d = nc.dram_tensor("out", (BSH, DIN), F32, kind="ExternalOutput").ap()

    rebase_at = _rebase_steps(steps)

    with tile.TileContext(nc) as tc, ExitStack() as top:
        thp = top.enter_context(tc.tile_pool(name="thp", bufs=1))
        nth_t = thp.tile([P, IT], F32)
        pth_t = thp.tile([P, IT], F32)
        eye2_t = thp.tile([P, 2, P], F8)
        nc.sync.dma_start(nth_t[:], nth_d.rearrange("(io p) -> p io", p=P))
        nc.sync.dma_start(pth_t[:], pth_d.rearrange("(io p) -> p io", p=P))
        nc.sync.dma_start(eye2_t[:], eye2_d)

        wpool = top.enter_context(tc.tile_pool(name="wpool", bufs=1))
        spool = top.enter_context(tc.tile_pool(name="spool", bufs=1))
        dxpool = top.enter_context(tc.tile_pool(name="dxpool", bufs=1))
        ypool = top.enter_context(tc.tile_pool(name="ypool", bufs=2))
        upool = top.enter_context(tc.tile_pool(name="upool", bufs=2))
        dpool = top.enter_context(tc.tile_pool(name="dpool", bufs=2))
        vhlp = top.enter_context(tc.tile_pool(name="vhlp", bufs=1))
        rbu = top.enter_context(tc.tile_pool(name="rbu", bufs=1))
        rbl = top.enter_context(tc.tile_pool(name="rbl", bufs=1))
        rbc = top.enter_context(tc.tile_pool(name="rbc", bufs=1))
        xp = top.enter_context(tc.tile_pool(name="xp", bufs=6))
        xb = top.enter_context(tc.tile_pool(name="xb", bufs=8))
        cp = top.enter_context(tc.tile_pool(name="cp", bufs=10))
        stC = top.enter_context(tc.tile_pool(name="stC", bufs=4))
        psX = top.enter_context(tc.tile_pool(name="psX", bufs=1, space="PSUM"))

        # y chunk 0 + W first: phase A can start while S8/S8lo/Dx stream in
        y_tiles = []
        for c in range(NCH):
            y_tiles.append(ypool.tile([P, KW, CH], F16, tag="y", name=f"y_{c}"))
        for ko in range(KW):
            nc.sync.dma_start(y_tiles[0][:, ko, :], yT_d[ts(ko, P), ds(0, CH)])
        W_t = wpool.tile([P, KW, DD], F16, name="W_t")
        for ko in range(KW):
            nc.sync.dma_start(W_t[:, ko, :], W_d[ts(ko, P), :])
        S8_t = spool.tile([P, IT, DD], F8, name="S8_t")
        for jo in range(IT):
            nc.sync.dma_start(S8_t[:, jo, :], S8_d[ts(jo, P), :])
        S8lo_t = None
        if rebase_at:
            S8lo_t = spool.tile([P, IT, DD], F8, name="S8lo_t")
            for jo in range(IT):
                nc.sync.dma_start(S8lo_t[:, jo, :], S8lo_d[ts(jo, P), :])
        Dx_t = dxpool.tile([P, IT, DIN], F16, name="Dx_t")
        for io in range(IT):
            nc.sync.dma_start(Dx_t[:, io, :], Dx_d[ts(io, P), :])

        def mm(out_ap, lhsT, rhs, stop, perf_mode=None, start=False):
            nc.tensor.matmul(out_ap, lhsT, rhs, start=start, stop=stop,
                             perf_mode=perf_mode, skip_group_check=True)

        def sweep_schedule(defer_groups=DEFER_G, defer_pairs=DEFER_PAIRS,
                           shift=DEFER_SHIFT):
            """Emission sequence [(g, pair)] where the FIRST defer_groups
            groups' high pairs are pushed `shift` group-slots later.  This
            delays the sweep's earliest demand for the last rhs tiles WITHOUT
            delaying late groups' completions (which gate the next
            elementwise sweep).  Returns (seq, completion-ordered groups)."""
            seq, backlog = [], {}
            for g in range(IT):
                seq.extend(backlog.pop(g, []))
                for p_ in range(NPAIR):
                    if g < defer_groups and p_ in defer_pairs:
                        backlog.setdefault(g + shift, []).append((g, p_))
                    else:
                        seq.append((g, p_))
            for s in sorted(backlog):
                seq.extend(backlog[s])
            lastpos = {}
            for idx, (g, p_) in enumerate(seq):
                lastpos[g] = idx
            order = sorted(range(IT), key=lambda g: lastpos[g])
            return seq, order

        SWEEP_SEQ, SWEEP_ORDER = sweep_schedule()

        def emit_sweep(X, S_t, rhs, inject=None, fresh=False):
            """DR sweep: X[g] (+)= S-block.T-contract rhs per SWEEP_SEQ.

            fresh=True: rewrite X in place -- even group's first matmul uses
            start=True, which marks the whole 2KB PSUM bank (both slices)
            pending-zero, so each slice's first matmul overwrites (a free
            memset) and the rest accumulate.  inject: v0hl tile whose (h,l)
            fp8 pair opens each group via eye2 (DoubleRow)."""
            started = set()
            for (g, p_) in SWEEP_SEQ:
                if g not in started:
                    started.add(g)
                    if inject is not None:
                        mm(X[g], eye2_t[:], inject[:, g, :, :], stop=False,
                           perf_mode=DR, start=(fresh and g % 2 == 0))
                        st_own = False
                    else:
                        st_own = fresh and g % 2 == 0
                else:
                    st_own = False
                mm(X[g], S_t[:, 2 * p_:2 * p_ + 2, ts(g, P)],
                   rhs[:, 2 * p_:2 * p_ + 2, :], stop=(p_ == NPAIR - 1),
                   perf_mode=DR, start=st_own)

        for c in range(NCH):
            if c + 1 < NCH:
                for ko in range(KW):
                    nc.sync.dma_start(y_tiles[c + 1][:, ko, :],
                                      yT_d[ts(ko, P), ds((c + 1) * CH, CH)])
            y_c = y_tiles[c]

            # -------- phase A: X[:,i,:] = 1024*v0 (fp16 accumulation) --------
            # Single [P, IT, CH] f32 tile = exactly all 8 PSUM banks; each
            # [P, CH] slice is half-bank-aligned so matmuls never cross banks.
            # Even group's first matmul start=True = free memset of its bank.
            X_t = psX.tile([P, IT, CH], F32, tag="psX", name="X_t")
            X = [X_t[:, i, :] for i in range(IT)]
            for i in range(IT):
                for ko in range(KW):
                    mm(X[i], W_t[:, ko, ts(i, P)], y_c[:, ko, :],
                       stop=(ko == KW - 1), start=(ko == 0 and i % 2 == 0))

            # -------- steps: fresh (t<rb), rebase (t=rb), delta tail, final a
            rb_t = rebase_at[0] if rebase_at else None
            u_prev = None
            v0hl = None
            for t in range(1, steps + 2):
                last = (t == steps + 1)
                rb = (t == rb_t)
                fresh = (rb_t is not None) and (t < rb_t)
                delta = not (last or rb or fresh)
                u16 = None
                if rb or delta or last:
                    u16 = upool.tile([P, IT, CH], F16, tag="u",
                                     name="a" if last else "u")
                u8 = None if not fresh else dpool.tile([P, IT, CH], F8, tag="d")
                d8 = None if not delta else dpool.tile([P, IT, CH], F8, tag="d")
                if t == 1:
                    v0hl = vhlp.tile([P, IT, 2, CH], F8, tag="vhl")
                # Early tiles (0..11): ONE ACT per PAIR of psum slices
                # ([128,2,256] = one bank) -- amortizes the 143ns psum access
                # and shrinks the ACT queue from 6.4us to 5.2us so the last
                # tiles' x-reads are completion-bound, not queue-bound.  Last
                # 4 tiles stay individual (shortest gating chain).  The
                # combine op of late tiles goes to DVE (faster than POOL).
                def x_aps():
                    for b in range(IT // 2 - 2):
                        x_b = xb.tile([P, 2, CH], F16, tag="xb")
                        nc.scalar.activation(x_b[:], X_t[:, 2 * b:2 * b + 2, :],
                                             IDENT, bias=0.0, scale=1.0 / SSC)
                        yield 2 * b, x_b[:, 0, :]
                        yield 2 * b + 1, x_b[:, 1, :]
                    for i_ in range(IT - 4, IT):
                        x_s = xp.tile([P, CH], F16, tag="x")
                        nc.scalar.activation(x_s[:], X[i_], IDENT, bias=0.0,
                                             scale=1.0 / SSC)
                        yield i_, x_s[:]
                for pos, (i, x_t) in enumerate(x_aps()):
                    if t == 1:
                        nc.vector.tensor_scalar_add(v0hl[:, i, 0, :], x_t, 0.0)
                        eng = nc.gpsimd if i % 2 == 0 else nc.vector
                        eng.tensor_tensor(v0hl[:, i, 1, :], x_t,
                                          v0hl[:, i, 0, :], SUB)
                    c_t = cp.tile([P, CH], F16, tag="c")
                    nc.vector.tensor_scalar(c_t[:], x_t, nth_t[:, i:i + 1],
                                            pth_t[:, i:i + 1], MAX, MIN)
                    if fresh:
                        # u8 = fp8(x - c) directly, split POOL/DVE
                        eng = nc.vector if pos >= DVE_POS else nc.gpsimd
                        eng.tensor_tensor(u8[:, i, :], x_t, c_t[:], SUB)
                    else:
                        nc.vector.tensor_tensor(u16[:, i, :], x_t, c_t[:], SUB)
                        if delta:
                            if u_prev is None:   # steps<8: first step is a cast
                                nc.vector.tensor_scalar_add(d8[:, i, :],
                                                            u16[:, i, :], 0.0)
                            else:
                                eng = nc.vector if pos >= 11 else nc.gpsimd
                                eng.tensor_tensor(d8[:, i, :], u16[:, i, :],
                                                  u_prev[:, i, :], SUB)
                if fresh:
                    emit_sweep(X, S8_t, u8, inject=v0hl, fresh=True)
                elif delta:
                    emit_sweep(X, S8_t, d8)
                elif rb:
                    # rebase: fresh X = v0(pair) + u(pair)@S8 + (2u)@(16R)
                    uh = rbu.tile([P, IT, CH], F8, tag="uh")
                    ul = rbl.tile([P, IT, CH], F8, tag="ul")
                    c8 = rbc.tile([P, IT, CH], F8, tag="c8")
                    for i in range(IT):
                        nc.vector.tensor_scalar_add(uh[:, i, :], u16[:, i, :], 0.0)
                        nc.gpsimd.tensor_tensor(ul[:, i, :], u16[:, i, :],
                                                uh[:, i, :], SUB)
                        nc.vector.tensor_scalar_mul(c8[:, i, :], u16[:, i, :],
                                                    1.0 / 16.0)
                    emit_sweep(X, S8_t, uh, inject=v0hl, fresh=True)
                    emit_sweep(X, S8_t, ul)
                    emit_sweep(X, S8lo_t, c8)
                if u16 is not None:
                    u_prev = u16

            # -------- phase C: out_chunk = (32a) @ (Dx/32), Dx resident ------
            for dn in range(DIN // CN):
                for bt in range(CH // P):
                    q = dn * (CH // P) + bt
                    ps = X[q]                     # reuse freed X slice as psum
                    for io in range(IT):
                        mm(ps, u_prev[:, io, ts(bt, P)],
                           Dx_t[:, io, ds(dn * CN, CN)], stop=(io == IT - 1),
                           start=(io == 0 and q % 2 == 0))
                    st = stC.tile([P, CN], F32, tag="stC")
                    nc.scalar.activation(st[:], ps, IDENT, bias=0.0, scale=1.0)
                    nc.sync.dma_start(out_d[ds(c * CH + bt * P, P),
                                            ds(dn * CN, CN)], st[:])

    nc.compile()
    return nc


def _prep_in_maps(y, W, Theta, S, Dx):
    y = np.ascontiguousarray(np.asarray(y, dtype=np.float32))
    W = np.asarray(W, dtype=np.float32)
    Theta = np.asarray(Theta, dtype=np.float32)
    S = np.asarray(S, dtype=np.float32)
    Dx = np.asarray(Dx, dtype=np.float32)
    assert y.shape == (B_FULL, DIN) and W.shape == (DIN, DD)
    assert S.shape == (DD, DD) and Dx.shape == (DD, DIN)

    W1024 = (W * np.float32(SC * SSC)).astype(np.float16)
    S8 = (S * np.float32(SSC)).astype(ml_dtypes.float8_e4m3)
    R = S * np.float32(SSC) - S8.astype(np.float32)
    S8lo = (16.0 * R).astype(ml_dtypes.float8_e4m3)
    Dx32 = (Dx / np.float32(SC)).astype(np.float16)
    th = np.maximum(Theta, 0.0) + np.float32(1e-7)
    nth32 = (-SC * th).astype(np.float32)
    pth32 = (SC * th).astype(np.float32)
    eye32 = (np.eye(P, dtype=np.float32) * 32.0).astype(ml_dtypes.float8_e4m3)
    eye2 = np.ascontiguousarray(np.stack([eye32, eye32], axis=1))  # (P, 2, P)
    yT = np.ascontiguousarray(y.T).astype(np.float16)   # [DIN, B]

    shared = dict(W1024=W1024, S8=S8, S8lo=S8lo, Dx32=Dx32,
                  nth32=nth32, pth32=pth32, eye2=eye2)
    in_maps = []
    for c in range(NCORES):
        sl = slice(c * BSH, (c + 1) * BSH)
        in_maps.append(dict(shared, yT=np.ascontiguousarray(yT[:, sl])))
    return in_maps


_sharded_cache = {}


def _get_sharded(steps: int):
    """Build (once) the jitted shard_map executable for the compiled NEFF."""
    if steps in _sharded_cache:
        return _sharded_cache[steps]
    import jax
    from jax.experimental.shard_map import shard_map
    from jax.sharding import Mesh, PartitionSpec
    from concourse import bass2jax

    if steps not in _built:
        _built[steps] = _build(steps)
    nc = _built[steps]
    bass2jax.install_neuronx_cc_hook()
    assert nc.dbg_addr is None
    partition_name = nc.partition_id_tensor.name if nc.partition_id_tensor else None

    in_names, out_names, out_avals, zero_shapes = [], [], [], []
    for alloc in nc.m.functions[0].allocations:
        if not isinstance(alloc, mybir.MemoryLocationSet):
            continue
        name = alloc.memorylocations[0].name
        if alloc.kind == "ExternalInput":
            if name != partition_name:
                in_names.append(name)
        elif alloc.kind == "ExternalOutput":
            out_names.append(name)
            shape = tuple(alloc.tensor_shape)
            dtype = mybir.dt.np(alloc.dtype)
            out_avals.append(jax.core.ShapedArray(shape, dtype))
            zero_shapes.append((shape, dtype))
    n_params = len(in_names)
    n_outs = len(out_names)
    all_in_names = in_names + out_names
    if partition_name is not None:
        all_in_names.append(partition_name)

    def _body(*args):
        operands = list(args)
        if partition_name is not None:
            operands.append(bass2jax.partition_id_tensor())
        outs = bass2jax._bass_exec_p.bind(
            *operands,
            out_avals=tuple(out_avals),
            in_names=tuple(all_in_names),
            out_names=tuple(out_names),
            lowering_input_output_aliases=(),
            sim_require_finite=True,
            sim_require_nnan=True,
            nc=nc,
        )
        return tuple(outs)

    devices = jax.devices()[:NCORES]
    mesh = Mesh(np.asarray(devices), ("core",))
    donate = tuple(range(n_params, n_params + n_outs))
    sharded = jax.jit(
        shard_map(_body, mesh=mesh,
                  in_specs=(PartitionSpec("core"),) * (n_params + n_outs),
                  out_specs=(PartitionSpec("core"),) * n_outs,
                  check_rep=False),
        donate_argnums=donate, keep_unused=True)
    entry = dict(sharded=sharded, in_names=in_names, out_names=out_names,
                 zero_shapes=zero_shapes, mesh=mesh, n_params=n_params)
    _sharded_cache[steps] = entry
    return entry


def _concat_inputs(entry, in_maps):
    return [np.concatenate([np.asarray(in_maps[c][n]) for c in range(NCORES)], axis=0)
            for n in entry["in_names"]]


def _run(entry, concat_in):
    zeros = [np.zeros((NCORES * s[0], *s[1:]), d) for s, d in entry["zero_shapes"]]
    out_arrs = entry["sharded"](*concat_in, *zeros)
    return out_arrs


def kernel(y, W, Theta, S, Dx, unroll_steps):
    steps = int(unroll_steps)
    entry = _get_sharded(steps)
    in_maps = _prep_in_maps(y, W, Theta, S, Dx)
    out_arrs = _run(entry, _concat_inputs(entry, in_maps))
    idx = entry["out_names"].index("out")
    return np.ascontiguousarray(np.asarray(out_arrs[idx]))  # [NCORES*BSH, DIN]


def time_kernel(np_inputs, iters=6):
    """Steady-state wall time per NEFF execution (ns), device-resident inputs."""
    import jax
    from jax.sharding import NamedSharding, PartitionSpec
    steps = int(np_inputs["unroll_steps"])
    entry = _get_sharded(steps)
    in_maps = _prep_in_maps(np_inputs["y"], np_inputs["W"], np_inputs["Theta"],
                            np_inputs["S"], np_inputs["Dx"])
    concat_in = _concat_inputs(entry, in_maps)
    sh = NamedSharding(entry["mesh"], PartitionSpec("core"))
    dev_in = [jax.device_put(a, sh) for a in concat_in]
    import time as _time
    times = []
    for it in range(iters):
        zeros = [jax.device_put(np.zeros((NCORES * s[0], *s[1:]), d), sh)
                 for s, d in entry["zero_shapes"]]
        for z in zeros:
            z.block_until_ready()
        t0 = _time.perf_counter()
        outs = entry["sharded"](*dev_in, *zeros)
        for o in outs:
            o.block_until_ready()
        times.append(_time.perf_counter() - t0)
    best = min(times[1:]) if len(times) > 1 else times[0]
    print("  per-iter times (ms):", [f"{t*1e3:.1f}" for t in times])
    return best * 1e9


if __name__ == "__main__":
    rng = np.random.default_rng(0)
    inputs = dict(
        y=rng.standard_normal((B_FULL, DIN), dtype=np.float32),
        W=(rng.standard_normal((DIN, DD)) * 0.02).astype(np.float32),
        Theta=rng.random(DD, dtype=np.float32),
        S=(rng.standard_normal((DD, DD)) * 0.02).astype(np.float32),
        Dx=(rng.standard_normal((DD, DIN)) * 0.02).astype(np.float32),
        unroll_steps=16,
    )
    out = kernel(**inputs)
    print("out", out.shape, out.dtype, np.abs(out).max())
